# revision 24
# baseline (speedup 1.0000x reference)
"""Trainium2 Bass kernel for nn_FP8Experts (MoE with FP8 block-quantized experts).

Strategy (expert-parallel over 8 NeuronCores):
  - Host: route tokens to experts by top_k_index (each expert's token list,
    padded to a common capacity C), pre-transpose + pre-halve the fp8 weights
    (OCP e4m3fn values > 240 are Inf/NaN in TRN e4m3; halving maps the value
    range exactly onto TRN fp8, compensated by doubling the dequant scale).
  - Device (per core = one expert): on-chip act-quant (per-token, per-128-block
    fp8 round-trip matching the reference), fp16 dequantized weights resident in
    SBUF, fp16 matmuls (gate_up -> silu*up -> act-quant -> down) accumulated in
    PSUM fp32. Activation transposes (contraction-major layout for the PE) run
    on the tensor engine itself to keep it HAM-warm.
  - Host: weighted combine with top_k_weights.

The activation fp8 round-trip also uses a /2-scaled grid (224 = 448/2) so TRN
e4m3 rounding reproduces OCP e4m3fn rounding bit-exactly (away from the
denormal floor, where the difference is ~2^-11 relative to the block amax).
"""

import numpy as np
import ml_dtypes

E, H, I = 8, 2048, 1408
T, TOPK = 4096, 2
BN = BK = 128
NCORES = 8
P = 128
HALF_MAX = 224.0

F8 = ml_dtypes.float8_e4m3  # TRN-compatible (bias 7, max 240)

_compiled_cache = {}
_weights_cache = {}


def _build(C):
    """Build + schedule the per-core Bass kernel for token capacity C."""
    import concourse.bass as bass
    import concourse.mybir as mybir
    import concourse.tile as tile
    from concourse import bacc
    from concourse.masks import make_identity

    f32 = mybir.dt.float32
    f16 = mybir.dt.float16
    f8 = mybir.dt.float8e4
    AF = mybir.ActivationFunctionType
    ALU = mybir.AluOpType
    AX = mybir.AxisListType

    NT = C // P
    KB1 = H // BK       # 16 contraction blocks for gate_up
    KB2 = I // BK       # 11 contraction blocks for down
    NB1 = 2 * I // BN   # 22 output blocks of gate_up
    NB2 = H // BN       # 16 output blocks of down

    nc = bacc.Bacc("TRN2", target_bir_lowering=False, debug=False,
                   num_devices=NCORES)

    x_d = nc.dram_tensor("x", [C, H], f32, kind="ExternalInput").ap()
    wgu_d = nc.dram_tensor("wgu16", [KB1, P, 2 * I], f16, kind="ExternalInput").ap()
    sgu_d = nc.dram_tensor("sgu", [KB1, P, NB1], f32, kind="ExternalInput").ap()
    wd_d = nc.dram_tensor("wd16", [KB2, P, H], f16, kind="ExternalInput").ap()
    sd_d = nc.dram_tensor("sd", [KB2, P, NB2], f32, kind="ExternalInput").ap()
    y_d = nc.dram_tensor("y", [C, H], f32, kind="ExternalOutput").ap()

    with tile.TileContext(nc) as tc:
        with (
            tc.tile_pool(name="const", bufs=1) as const,
            tc.tile_pool(name="wpool", bufs=1) as wpool,
            tc.tile_pool(name="stage", bufs=3) as stage,
            tc.tile_pool(name="xio", bufs=2) as xio,
            tc.tile_pool(name="qp", bufs=2) as qp,
            tc.tile_pool(name="tp", bufs=2) as tp,
            tc.tile_pool(name="pp", bufs=5, space="PSUM") as pp,
            tc.tile_pool(name="pt", bufs=2, space="PSUM") as pt,
            tc.tile_pool(name="ka", bufs=1, space="PSUM") as ka,
        ):
            ident = const.tile([P, P], f16, name="ident")
            make_identity(nc, ident[:])

            # PE warmup: dense dummy matmuls so the HAM clock-gate is at
            # 8/8 (2.4 GHz) by the time the first real matmul issues.
            ps_warm = ka.tile([P, 512], f32, name="ps_warm", tag="ps_warm")
            for _ in range(120):
                nc.tensor.matmul(ps_warm[:, :P], lhsT=ident[:], rhs=ident[:],
                                 start=True, stop=True)

            # ---------------- weight dequant (once, chunk-granular) --------
            wgu_all = wpool.tile([P, KB1, 2 * I], f16, name="wgu_all")
            wd_all = wpool.tile([P, KB2, H], f16, name="wd_all")
            wgu16 = [wgu_all[:, kb, :] for kb in range(KB1)]
            wd16 = [wd_all[:, kb, :] for kb in range(KB2)]

            scg32 = const.tile([P, KB1, NB1], f32, name="scg32")
            nc.sync.dma_start(scg32[:], sgu_d.rearrange("k p n -> p k n"))
            scd32 = const.tile([P, KB2, NB2], f32, name="scd32")
            nc.sync.dma_start(scd32[:], sd_d.rearrange("k p n -> p k n"))
            scg = const.tile([P, KB1, NB1], f16, name="scg")
            nc.vector.tensor_copy(out=scg[:], in_=scg32[:])
            scd = const.tile([P, KB2, NB2], f16, name="scd")
            nc.vector.tensor_copy(out=scd[:], in_=scd32[:])

            # prefetch tile 0's tokens ahead of the weight-stage DMAs
            xt0 = xio.tile([P, H], f32, name="xt", tag="xt")
            nc.sync.dma_start(xt0[:], x_d[0:P, :])

            # rate-balanced DVE/ACT/GPSIMD split of the dequant multiplies
            eng_time = {"D": 0.0, "A": 0.0, "G": 0.0}

            def dequant_quad(q0, qn, c0, cw, w_dram, sc16, sc32, out_all):
                """Dequant cols [c0,c0+cw) of contraction-tiles [q0,q0+qn)."""
                nb = cw // BN
                b0 = c0 // BN
                wst = stage.tile([P, 4, 512], f16, name="wst",
                                 tag="wst")[:, :qn, :cw]
                nc.sync.dma_start(
                    wst, w_dram[q0:q0 + qn, :, c0:c0 + cw].rearrange(
                        "k p n -> p k n"))
                # measured per-unit costs (ns): DVE 1x, ACT ~520/op, GP ~2x DVE
                costs = {"D": 60 + qn * cw * 1.07,
                         "A": qn * nb * 520.0,
                         "G": 1000 + qn * cw * 2.2}
                eng = min(costs, key=lambda k: eng_time[k] + costs[k])
                eng_time[eng] += costs[eng]
                if eng == "A":
                    for q in range(qn):
                        for b in range(nb):
                            nc.scalar.activation(
                                out_all[:, q0 + q,
                                        c0 + b * BN:c0 + (b + 1) * BN],
                                wst[:, q, b * BN:(b + 1) * BN],
                                AF.Copy,
                                scale=sc32[:, q0 + q, b0 + b:b0 + b + 1],
                            )
                else:
                    e = nc.vector if eng == "D" else nc.gpsimd
                    e.tensor_tensor(
                        out=out_all[:, q0:q0 + qn, c0:c0 + cw].rearrange(
                            "p q (b n) -> p q b n", n=BN),
                        in0=wst.rearrange("p q (b n) -> p q b n", n=BN),
                        in1=sc16[:, q0:q0 + qn, b0:b0 + nb, None]
                        .to_broadcast([P, qn, nb, BN]),
                        op=ALU.mult,
                    )

            # chunk-major, in matmul pair order (g0,u0,g1,u1,g2,u2) so the
            # first matmul chunks' inputs finish first
            GU_CH = [(0, 512), (1408, 512), (512, 512), (1920, 512),
                     (1024, 384), (2432, 384)]
            for (c0, cw) in GU_CH:
                for q0 in range(0, KB1, 4):
                    dequant_quad(q0, 4, c0, cw, wgu_d, scg, scg32, wgu_all)
            for c0 in range(0, H, 512):
                for q0 in range(0, KB2, 4):
                    dequant_quad(q0, min(4, KB2 - q0), c0, 512, wd_d, scd,
                                 scd32, wd_all)

            # gate/up paired column chunks: (offset-within-half, width, #blocks)
            GCHUNKS = [(0, 512, 4), (512, 512, 4), (1024, 384, 3)]

            def keepalive(n):
                """Dummy matmuls that hold the HAM clock-gate open while the
                PE would otherwise idle waiting on ramp-phase dequant."""
                for _ in range(n):
                    nc.tensor.matmul(ps_warm[:, :P], lhsT=ident[:],
                                     rhs=ident[:], start=True, stop=True)

            def pe_transpose(src, dst, nblk):
                """[token, feat] -> [feat, token] via PE, 4 blocks per bank."""
                for g0 in range(0, nblk, 4):
                    gn = min(4, nblk - g0)
                    ps_t = pt.tile([P, 4, P], f16, name="ps_t", tag="ps_t")
                    for j in range(gn):
                        nc.tensor.transpose(ps_t[:, j, :], src[:, g0 + j, :],
                                            ident[:])
                    nc.vector.tensor_copy(out=dst[:, g0:g0 + gn, :],
                                          in_=ps_t[:, :gn, :])

            # ---------------- main loop over 128-token tiles ----------------
            for tt in range(NT):
                if tt == 0:
                    xt = xt0
                else:
                    xt = xio.tile([P, H], f32, name="xt", tag="xt")
                    nc.sync.dma_start(xt[:], x_d[tt * P:(tt + 1) * P, :])

                # --- act quant of x: per-token per-128-block fp8 roundtrip ---
                amax = qp.tile([P, KB1], f32, name="amax", tag="amax")
                nc.vector.reduce_max(
                    amax[:], xt.rearrange("p (b k) -> p b k", k=BK),
                    axis=AX.X, apply_absolute_value=True,
                )
                inv = qp.tile([P, KB1], f32, name="inv", tag="inv")
                s2 = qp.tile([P, KB1], f32, name="s2", tag="s2")
                nc.vector.tensor_scalar_max(amax[:], amax[:], 1e-12)
                nc.vector.reciprocal(inv[:], amax[:])
                nc.vector.tensor_scalar_mul(inv[:], inv[:], HALF_MAX)
                nc.vector.tensor_scalar_mul(s2[:], amax[:], 1.0 / HALF_MAX)

                q8 = qp.tile([P, KB1, BK], f8, name="q8", tag="q8", bufs=1)
                nc.vector.tensor_tensor(
                    out=q8[:],
                    in0=xt.rearrange("p (b k) -> p b k", k=BK),
                    in1=inv[:, :, None].to_broadcast([P, KB1, BK]),
                    op=ALU.mult,
                )
                xq16 = qp.tile([P, KB1, BK], f16, name="xq16", tag="xq16")
                nc.vector.tensor_tensor(
                    out=xq16[:], in0=q8[:],
                    in1=s2[:, :, None].to_broadcast([P, KB1, BK]),
                    op=ALU.mult,
                )

                # --- transpose xq to contraction-major [feat, token] ---
                xqT = qp.tile([P, KB1, BK], f16, name="xqT", tag="xqT")
                pe_transpose(xq16, xqT, KB1)

                # --- gate_up matmul + silu*up + act quant of inter ---
                iq16 = qp.tile([P, KB2, BK], f16, name="iq16", tag="iq16")
                amax_i = qp.tile([P, KB2], f32, name="amax_i", tag="amax_i")
                inv_i = qp.tile([P, KB2], f32, name="inv_i", tag="inv_i")
                s2_i = qp.tile([P, KB2], f32, name="s2_i", tag="s2_i")

                for (off, w, nb) in GCHUNKS:
                    if tt < 3:
                        keepalive(6)
                    ps_g = pp.tile([P, 512], f32, name="ps", tag="ps")[:, :w]
                    for kb in range(KB1):
                        nc.tensor.matmul(ps_g, lhsT=xqT[:, kb, :],
                                         rhs=wgu16[kb][:, off:off + w],
                                         start=(kb == 0), stop=(kb == KB1 - 1))
                    ps_u = pp.tile([P, 512], f32, name="ps", tag="ps")[:, :w]
                    for kb in range(KB1):
                        nc.tensor.matmul(ps_u, lhsT=xqT[:, kb, :],
                                         rhs=wgu16[kb][:, I + off:I + off + w],
                                         start=(kb == 0), stop=(kb == KB1 - 1))
                    sil = tp.tile([P, 512], f32, name="sil", tag="sil")[:, :w]
                    nc.scalar.activation(sil, ps_g, AF.Silu)
                    itc = tp.tile([P, 512], f32, name="itc", tag="itc")[:, :w]
                    nc.vector.tensor_mul(itc, sil, ps_u)

                    b0 = off // BN
                    am = amax_i[:, b0:b0 + nb]
                    nc.vector.reduce_max(
                        am, itc.rearrange("p (b k) -> p b k", k=BK),
                        axis=AX.X, apply_absolute_value=True,
                    )
                    nc.vector.tensor_scalar_max(am, am, 1e-12)
                    nc.vector.reciprocal(inv_i[:, b0:b0 + nb], am)
                    nc.vector.tensor_scalar_mul(inv_i[:, b0:b0 + nb],
                                                inv_i[:, b0:b0 + nb], HALF_MAX)
                    nc.vector.tensor_scalar_mul(s2_i[:, b0:b0 + nb], am,
                                                1.0 / HALF_MAX)
                    qi8 = tp.tile([P, 512], f8, name="qi8", tag="qi8")[:, :w]
                    nc.vector.tensor_tensor(
                        out=qi8.rearrange("p (b k) -> p b k", k=BK),
                        in0=itc.rearrange("p (b k) -> p b k", k=BK),
                        in1=inv_i[:, b0:b0 + nb, None].to_broadcast(
                            [P, nb, BK]),
                        op=ALU.mult,
                    )
                    nc.vector.tensor_tensor(
                        out=iq16[:, b0:b0 + nb, :],
                        in0=qi8.rearrange("p (b k) -> p b k", k=BK),
                        in1=s2_i[:, b0:b0 + nb, None].to_broadcast(
                            [P, nb, BK]),
                        op=ALU.mult,
                    )

                iqT = qp.tile([P, KB2, BK], f16, name="iqT", tag="iqT")
                pe_transpose(iq16, iqT, KB2)

                # --- down matmul + store ---
                for hc in range(4):
                    if tt < 3:
                        keepalive(6)
                    ps_y = pp.tile([P, 512], f32, name="ps", tag="ps")
                    for kb in range(KB2):
                        nc.tensor.matmul(ps_y, lhsT=iqT[:, kb, :],
                                         rhs=wd16[kb][:, hc * 512:(hc + 1) * 512],
                                         start=(kb == 0), stop=(kb == KB2 - 1))
                    yt = tp.tile([P, 512], f32, name="yt", tag="yt")
                    nc.scalar.copy(yt[:], ps_y[:])
                    nc.sync.dma_start(
                        y_d[tt * P:(tt + 1) * P, hc * 512:(hc + 1) * 512], yt[:])

    nc.compile()
    return nc


def _prep_weights(gate_up_proj, gate_up_proj_scale_inv, down_proj,
                  down_proj_scale_inv):
    """Per-expert transposed fp8 weights upcast (bit-exact) to fp16, plus
    fp16 broadcast scales. The dequant multiply itself runs on-device."""
    key = (id(gate_up_proj), id(down_proj))
    if key in _weights_cache:
        return _weights_cache[key]
    KB1, KB2, NB1, NB2 = H // BK, I // BK, 2 * I // BN, H // BN
    out = []
    gup = np.asarray(gate_up_proj)
    gus = np.asarray(gate_up_proj_scale_inv, dtype=np.float32)
    dwn = np.asarray(down_proj)
    dws = np.asarray(down_proj_scale_inv, dtype=np.float32)
    for e in range(E):
        wgu16 = np.ascontiguousarray(
            gup[e].astype(np.float16).T).reshape(KB1, P, 2 * I)
        sgu = np.broadcast_to(gus[e].T[:, None, :], (KB1, P, NB1)).copy()
        wd16 = np.ascontiguousarray(
            dwn[e].astype(np.float16).T).reshape(KB2, P, H)
        sd = np.broadcast_to(dws[e].T[:, None, :], (KB2, P, NB2)).copy()
        out.append((wgu16, sgu, wd16, sd))
    _weights_cache[key] = out
    return out


def kernel(hidden_states, top_k_index, top_k_weights, gate_up_proj,
           gate_up_proj_scale_inv, down_proj, down_proj_scale_inv,
           _trace=False, _tmpdir=None):
    from concourse import bass_utils

    hs = np.ascontiguousarray(np.asarray(hidden_states, dtype=np.float32))
    tki = np.asarray(top_k_index)
    tkw = np.asarray(top_k_weights, dtype=np.float32)

    # ---- host routing (the "all-to-all dispatch") ----
    toks_per_e = []
    for e in range(E):
        toks_per_e.append(np.nonzero((tki == e).any(axis=1))[0])
    max_count = max(len(t) for t in toks_per_e)
    C = max(P, -(-max_count // P) * P)

    if C not in _compiled_cache:
        _compiled_cache[C] = _build(C)
    nc = _compiled_cache[C]

    wprep = _prep_weights(gate_up_proj, gate_up_proj_scale_inv, down_proj,
                          down_proj_scale_inv)

    in_maps = []
    for e in range(E):
        toks = toks_per_e[e]
        x = np.zeros((C, H), np.float32)
        x[:len(toks)] = hs[toks]
        wgu16, sgu, wd16, sd = wprep[e]
        in_maps.append({"x": x, "wgu16": wgu16, "sgu": sgu, "wd16": wd16,
                        "sd": sd})

    res = bass_utils.run_bass_kernel_spmd(
        nc, in_maps, core_ids=list(range(NCORES)),
        trace=_trace, tmpdir=_tmpdir,
    )

    # ---- host combine ----
    out = np.zeros((T, H), np.float32)
    for e in range(E):
        toks = toks_per_e[e]
        y = res.results[e]["y"]
        for kk in range(TOPK):
            sel = np.nonzero(tki[:, kk] == e)[0]
            pos = np.searchsorted(toks, sel)
            out[sel] += tkw[sel, kk, None] * y[pos]
    if _trace:
        kernel._last_results = res
    return out
